# revision 38
# baseline (speedup 1.0000x reference)
"""CLIP contrastive loss on 8 Trainium2 NeuronCores (Bass/Tile), fp8 edition.

Strategy (data-parallel over image rows, hint's local_loss path):
  - Core c holds image rows [c*1024, (c+1)*1024) and the FULL text matrix.
  - Text rows are rolled by c*1024 on the host so every core's diagonal
    block sits at local column 0 (the compiled program is core-independent).
  - Features are pre-scaled by 16 and quantized to fp8 e4m3 on the host;
    matmuls run in DoubleRow perf mode (2 fp8 MACs/cell/cycle), K=512 as
    two K=256 DoubleRow accumulation steps.  Loop order is mt-outer /
    column-group-inner so the stationary (image) operand is reused across
    8 matmuls per LDWEIGHTS.
  - PSUM is split in two 4-bank groups [128,2048] f32 (ping-pong).  For
    each finished group, ScalarE does one exp activation (scale=100/256,
    bias=-shift) PSUM->SBUF bf16.
  - VectorE folds each exp tile into a per-column-group accumulator with
    tensor_tensor_reduce; its free per-partition accum output yields the
    running (prefix over mt) sum R[mt,g] = sum_j colacc[p, g-cols].  The
    host recovers per-(mt,g) row-sum chunks by differencing R along mt --
    no ACTIVATION_READ_ACCUMULATOR instructions needed.
  - Host: diag computed exactly in f64 (tiny einsum); col sums reduced
    over partitions (+roll); row sums from R diffs; loss in f64.

Fixed-shift logsumexp is numerically safe: logits are bounded by +-scale
and shift = scale/2 keeps every term that matters in normal f32 range.
"""

from contextlib import ExitStack

import numpy as np
import ml_dtypes

import concourse.bass as bass
from concourse import bacc
import concourse.tile as tile
from concourse import mybir
from concourse.bass import ts
from concourse.bass_utils import run_bass_kernel_spmd

N = 8192
D = 512
NC = 8
M_LOC = N // NC          # 1024 image rows per core
MT = M_LOC // 128        # 8 m-tiles of 128 rows
NTT = N // 1024          # 8 text tiles of 1024 cols
NG = N // 2048           # 4 column groups of 2048 cols (one 4-bank PSUM set)
KC = D // 128            # 4 contraction chunks of 128
FSCALE = 16.0            # host feature pre-scale before fp8 quantization

F32 = mybir.dt.float32
BF16 = mybir.dt.bfloat16
FP8 = mybir.dt.float8e4
NP_FP8 = ml_dtypes.float8_e4m3   # IEEE e4m3: max 240, matches TRN FP8_EXP4

import os

USE_DR = os.environ.get("KMM_DR", "1") == "1"  # DoubleRow fp8 matmuls
# rowsum strategy: "act" = activation accum_out (safe),
# "ttr" = tensor_tensor_reduce prefix trick, "stt" = scalar_tensor_tensor prefix,
# "hybrid" = mix of act (direct chunks) and stt (prefix R) balancing ACT vs DVE
ROWMODE = os.environ.get("KROW", "hybrid")


TRICK_SET = (2, 7, 13, 18, 23, 27)  # spread out, none in mt=7 (shorter tail)


def _is_trick(ridx: int) -> bool:
    # groups whose exp runs on DVE via the exp2 bit-trick (frees ScalarE)
    return ROWMODE == "hybrid" and ridx in TRICK_SET


def _is_act_step(ridx: int) -> bool:
    if ROWMODE == "act":
        return True
    if ROWMODE in ("ttr", "stt"):
        return False
    return not _is_trick(ridx)  # hybrid: ACT accum everywhere except trick groups


LOG2E = 1.4426950408889634
C_MAGIC = 0.05753  # exp2 linear-interp magic, calibrated mean-neutral (RNE)

_CACHE = {}
LAST_RESULTS = None


def _build(scale: float, shift: float):
    act_scale = scale / (FSCALE * FSCALE)
    # DVE exp2 bit-trick: i16 bits = round(s*trick_a + trick_b) viewed as bf16
    # give 2^((l - shift)*log2e) = exp(l - shift), l = act_scale * s
    trick_a = 128.0 * LOG2E * act_scale
    trick_b = 128.0 * (127.0 - C_MAGIC - shift * LOG2E)
    nc = bacc.Bacc("TRN2", debug=False)

    at_d = nc.dram_tensor("at_in", [128, KC, M_LOC], FP8, kind="ExternalInput").ap()
    bt_d = nc.dram_tensor("bt_in", [NTT, 128, KC, 1024], FP8, kind="ExternalInput").ap()

    colsum_d = nc.dram_tensor("colsum_out", [128, N], BF16, kind="ExternalOutput").ap()
    rowr_d = nc.dram_tensor("rowr_out", [128, MT * NG], F32, kind="ExternalOutput").ap()

    with ExitStack() as ctx:
        tc = ctx.enter_context(tile.TileContext(nc))
        singles = ctx.enter_context(tc.tile_pool(name="singles", bufs=1))
        btp = ctx.enter_context(tc.tile_pool(name="btp", bufs=NTT))
        expp = ctx.enter_context(tc.tile_pool(name="expp", bufs=6))
        psum = ctx.enter_context(tc.tile_pool(name="psum", bufs=2, space="PSUM"))

        at_t = singles.tile([128, KC, M_LOC], FP8)
        bt_tiles = [
            btp.tile([128, KC, 1024], FP8, name=f"bt{t}", tag="bt")
            for t in range(NTT)
        ]
        # Parallel first loads across three DMA queues (sync/scalar/gpsimd).
        # Tiny head slices (mt0 weights + first 512 cols of bt0) land in
        # ~1us so the first matmul can issue early; remainders follow.
        nc.sync.dma_start(at_t[:, 0:2, 0:128], at_d[:, 0:2, 0:128])
        nc.scalar.dma_start(bt_tiles[0][:, 0:2, 0:512], bt_d[0, :, 0:2, 0:512])
        nc.scalar.dma_start(bt_tiles[0][:, 0:2, 512:], bt_d[0, :, 0:2, 512:])
        nc.sync.dma_start(at_t[:, 0:2, 128:], at_d[:, 0:2, 128:])
        nc.sync.dma_start(at_t[:, 2:4, :], at_d[:, 2:4, :])
        nc.scalar.dma_start(bt_tiles[0][:, 2:4, :], bt_d[0, :, 2:4, :])
        nc.gpsimd.dma_start(bt_tiles[1][:, 0:2, :], bt_d[1, :, 0:2, :])
        nc.gpsimd.dma_start(bt_tiles[1][:, 2:4, :], bt_d[1, :, 2:4, :])
        for t in range(2, NTT):
            eng = nc.sync if t % 2 == 0 else nc.gpsimd
            eng.dma_start(bt_tiles[t], bt_d[t])

        # PE pre-warm: a few dummy matmuls on a zeroed SBUF tile start the
        # HAM activity clock while the first input DMAs are in flight.
        warm_t = singles.tile([128, 2, 128], FP8)
        nc.vector.memset(warm_t, 0.0)
        warm_ps = psum.tile([128, 2048], F32, name="warmps", tag="spsum")
        for w in range(4):
            nc.tensor.matmul(
                warm_ps[:, ts(w, 512)][:, 0:128],
                warm_t,
                warm_t,
                start=True,
                stop=True,
                perf_mode=mybir.MatmulPerfMode.DoubleRow,
            )



        bias_t = singles.tile([128, 1], F32)
        nc.vector.memset(bias_t, -shift)
        colacc_a = singles.tile([128, N], BF16)
        colacc_b = singles.tile([128, N], BF16)
        if ROWMODE == "ttr":
            nc.vector.memset(colacc_b, 0.0)
        rowr_sb = singles.tile([128, MT * NG], F32)

        def emit_colacc(pend):
            mt, g, ridx, e_ap, act_step = pend
            dst, src = (
                (colacc_a, colacc_b) if mt % 2 == 0 else (colacc_b, colacc_a)
            )
            if ROWMODE == "ttr":
                nc.vector.tensor_tensor_reduce(
                    out=dst[:, ts(g, 2048)],
                    in0=src[:, ts(g, 2048)],
                    in1=e_ap,
                    scale=1.0,
                    scalar=0.0,
                    op0=mybir.AluOpType.add,
                    op1=mybir.AluOpType.add,
                    accum_out=rowr_sb[:, ridx : ridx + 1],
                )
            elif not act_step:
                # out = (e * s) + src ; accum_out = sum(out) = prefix R[mt,g]
                # mt==0: out = (e * 0) + e  (no src needed, no memset)
                nc.vector.scalar_tensor_tensor(
                    out=dst[:, ts(g, 2048)],
                    in0=e_ap,
                    scalar=1.0 if mt > 0 else 0.0,
                    in1=src[:, ts(g, 2048)] if mt > 0 else e_ap,
                    op0=mybir.AluOpType.mult,
                    op1=mybir.AluOpType.add,
                    accum_out=rowr_sb[:, ridx : ridx + 1],
                )
            elif mt == 0:
                nc.vector.tensor_copy(dst[:, ts(g, 2048)], e_ap)
            else:
                nc.vector.tensor_add(dst[:, ts(g, 2048)], src[:, ts(g, 2048)], e_ap)
            if mt == MT - 1:
                nc.sync.dma_start(colsum_d[:, ts(g, 2048)], dst[:, ts(g, 2048)])

        pending = None
        for mt in range(MT):
            for g in range(NG):
                s_ps = psum.tile([128, 2048], F32, name=f"s{mt}_{g}", tag="spsum")
                if USE_DR:
                    for kc2 in range(2):
                        for b in range(4):
                            t, h = 2 * g + b // 2, b % 2
                            nc.tensor.matmul(
                                s_ps[:, ts(b, 512)],
                                at_t[:, 2 * kc2 : 2 * kc2 + 2, ts(mt, 128)],
                                bt_tiles[t][:, 2 * kc2 : 2 * kc2 + 2, ts(h, 512)],
                                start=(kc2 == 0),
                                stop=(kc2 == 1),
                                perf_mode=mybir.MatmulPerfMode.DoubleRow,
                            )
                else:
                    for kc in range(KC):
                        for b in range(4):
                            t, h = 2 * g + b // 2, b % 2
                            nc.tensor.matmul(
                                s_ps[:, ts(b, 512)],
                                at_t[:, kc, ts(mt, 128)],
                                bt_tiles[t][:, kc, ts(h, 512)],
                                start=(kc == 0),
                                stop=(kc == KC - 1),
                            )
                ridx = mt * NG + g
                act_step = _is_act_step(ridx)
                if _is_trick(ridx):
                    # pure-float exp2 bit trick: y = s*a + (b + 1.5*2^23) in
                    # f32; RNE snaps y to an integer, whose low 16 bits equal
                    # round(s*a + b) = the bf16 bit pattern of exp(l - shift).
                    e_f = expp.tile(
                        [128, 2048, 2], BF16, name=f"e{mt}_{g}", tag="exp"
                    )
                    nc.vector.tensor_scalar(
                        out=e_f.bitcast(F32).squeeze(2),
                        in0=s_ps,
                        scalar1=trick_a,
                        scalar2=trick_b + 12582912.0,  # + 1.5 * 2^23
                        op0=mybir.AluOpType.mult,
                        op1=mybir.AluOpType.add,
                    )
                    e_t = e_f[:, :, 0]  # low halves, stride 2
                else:
                    e_t = expp.tile([128, 2048], BF16, name=f"e{mt}_{g}", tag="exp")
                    nc.scalar.activation(
                        e_t,
                        s_ps,
                        mybir.ActivationFunctionType.Exp,
                        bias=bias_t,
                        scale=act_scale,
                        accum_out=(
                            rowr_sb[:, ridx : ridx + 1] if act_step else None
                        ),
                    )
                if pending is not None:
                    emit_colacc(pending)
                pending = (mt, g, ridx, e_t, act_step)
        emit_colacc(pending)
        nc.sync.dma_start(rowr_d, rowr_sb)

    nc.compile()
    return nc


def _prep_inputs(img, txt):
    imgq = (FSCALE * img).astype(NP_FP8)
    txtq = (FSCALE * txt).astype(NP_FP8)
    in_maps = []
    for c in range(NC):
        A = imgq[c * M_LOC : (c + 1) * M_LOC]                   # [1024, 512] fp8
        at = np.ascontiguousarray(
            A.T.reshape(KC, 128, M_LOC).transpose(1, 0, 2)
        )                                                       # [128, 4, 1024]
        tr = np.roll(txtq, -c * M_LOC, axis=0)                  # local col j -> global (j + c*1024) % N
        bt = np.ascontiguousarray(
            tr.T.reshape(KC, 128, NTT, 1024).transpose(2, 1, 0, 3)
        )                                                       # [8, 128, 4, 1024]
        in_maps.append({"at_in": at, "bt_in": bt})
    return in_maps


def kernel(image_features, text_features, logit_scale):
    global LAST_RESULTS
    img = np.ascontiguousarray(np.asarray(image_features, dtype=np.float32))
    txt = np.ascontiguousarray(np.asarray(text_features, dtype=np.float32))
    scale = float(np.asarray(logit_scale))
    shift = 0.5 * scale

    if scale not in _CACHE:
        _CACHE[scale] = _build(scale, shift)
    nc = _CACHE[scale]

    in_maps = _prep_inputs(img, txt)
    res = run_bass_kernel_spmd(nc, in_maps, core_ids=list(range(NC)))
    LAST_RESULTS = res

    # exact diagonal in f64 (independent of the fp8 matmul path)
    diag = scale * np.einsum(
        "ij,ij->i", img.astype(np.float64), txt.astype(np.float64)
    )

    colsum_tot = np.zeros(N, dtype=np.float64)
    lse_rows = []
    for c, r in enumerate(res.results):
        colacc = r["colsum_out"].astype(np.float64)             # [128, N]
        colsum_tot += np.roll(colacc.sum(axis=0), c * M_LOC)
        R = r["rowr_out"].astype(np.float64).reshape(128, MT, NG)
        # act steps: R[mt,g] is the (mt,g) chunk directly.
        # stt steps: R[mt,g] is the prefix sum_{m<=mt} chunk[m,g]; recover
        # sequentially (works for any act/stt interleaving per g).
        chunks = np.empty_like(R)
        prefix = np.zeros((128, NG), dtype=np.float64)
        for mt in range(MT):
            for g in range(NG):
                if _is_act_step(mt * NG + g):
                    chunks[:, mt, g] = R[:, mt, g]
                else:
                    # measured bf16 prefix minus f32-exact prefix can go
                    # slightly negative for tiny chunks; true chunks are >= 0
                    chunks[:, mt, g] = np.maximum(R[:, mt, g] - prefix[:, g], 0.0)
                prefix[:, g] += chunks[:, mt, g]
        rowsum = chunks.sum(axis=2)                             # [p, mt]
        lse_rows.append(shift + np.log(rowsum.T.reshape(-1)))   # row = mt*128 + p
    lse_row = np.concatenate(lse_rows)
    lse_col = shift + np.log(colsum_tot)

    loss = 0.5 * (np.mean(lse_row - diag) + np.mean(lse_col - diag))
    return np.float32(loss)


# revision 39
# speedup vs baseline: 1.0224x; 1.0224x over previous
"""CLIP contrastive loss on 8 Trainium2 NeuronCores (Bass/Tile), fp8 edition.

Strategy (data-parallel over image rows, hint's local_loss path):
  - Core c holds image rows [c*1024, (c+1)*1024) and the FULL text matrix.
  - Text rows are rolled by c*1024 on the host so every core's diagonal
    block sits at local column 0 (the compiled program is core-independent).
  - Features are pre-scaled by 16 and quantized to fp8 e4m3 on the host;
    matmuls run in DoubleRow perf mode (2 fp8 MACs/cell/cycle), K=512 as
    two K=256 DoubleRow accumulation steps.  Loop order is mt-outer /
    column-group-inner so the stationary (image) operand is reused across
    8 matmuls per LDWEIGHTS.
  - PSUM is split in two 4-bank groups [128,2048] f32 (ping-pong).  For
    each finished group, ScalarE does one exp activation (scale=100/256,
    bias=-shift) PSUM->SBUF bf16.
  - VectorE folds each exp tile into a per-column-group accumulator with
    tensor_tensor_reduce; its free per-partition accum output yields the
    running (prefix over mt) sum R[mt,g] = sum_j colacc[p, g-cols].  The
    host recovers per-(mt,g) row-sum chunks by differencing R along mt --
    no ACTIVATION_READ_ACCUMULATOR instructions needed.
  - Host: diag computed exactly in f64 (tiny einsum); col sums reduced
    over partitions (+roll); row sums from R diffs; loss in f64.

Fixed-shift logsumexp is numerically safe: logits are bounded by +-scale
and shift = scale/2 keeps every term that matters in normal f32 range.
"""

from contextlib import ExitStack

import numpy as np
import ml_dtypes

import concourse.bass as bass
from concourse import bacc
import concourse.tile as tile
from concourse import mybir
from concourse.bass import ts
from concourse.bass_utils import run_bass_kernel_spmd

N = 8192
D = 512
NC = 8
M_LOC = N // NC          # 1024 image rows per core
MT = M_LOC // 128        # 8 m-tiles of 128 rows
NTT = N // 1024          # 8 text tiles of 1024 cols
NG = N // 2048           # 4 column groups of 2048 cols (one 4-bank PSUM set)
KC = D // 128            # 4 contraction chunks of 128
FSCALE = 16.0            # host feature pre-scale before fp8 quantization

F32 = mybir.dt.float32
BF16 = mybir.dt.bfloat16
FP8 = mybir.dt.float8e4
NP_FP8 = ml_dtypes.float8_e4m3   # IEEE e4m3: max 240, matches TRN FP8_EXP4

import os

USE_DR = os.environ.get("KMM_DR", "1") == "1"  # DoubleRow fp8 matmuls
# rowsum strategy: "act" = activation accum_out (safe),
# "ttr" = tensor_tensor_reduce prefix trick, "stt" = scalar_tensor_tensor prefix,
# "hybrid" = mix of act (direct chunks) and stt (prefix R) balancing ACT vs DVE
ROWMODE = os.environ.get("KROW", "hybrid")


TRICK_SET = (2, 7, 13, 18, 23, 27)  # spread out, none in mt=7 (shorter tail)


def _is_trick(ridx: int) -> bool:
    # groups whose exp runs on DVE via the exp2 bit-trick (frees ScalarE)
    return ROWMODE == "hybrid" and ridx in TRICK_SET


def _is_act_step(ridx: int) -> bool:
    if ROWMODE == "act":
        return True
    if ROWMODE in ("ttr", "stt"):
        return False
    return not _is_trick(ridx)  # hybrid: ACT accum everywhere except trick groups


LOG2E = 1.4426950408889634
C_MAGIC = 0.05753  # exp2 linear-interp magic, calibrated mean-neutral (RNE)

_CACHE = {}
LAST_RESULTS = None


def _build(scale: float, shift: float):
    act_scale = scale / (FSCALE * FSCALE)
    # DVE exp2 bit-trick: i16 bits = round(s*trick_a + trick_b) viewed as bf16
    # give 2^((l - shift)*log2e) = exp(l - shift), l = act_scale * s
    trick_a = 128.0 * LOG2E * act_scale
    trick_b = 128.0 * (127.0 - C_MAGIC - shift * LOG2E)
    nc = bacc.Bacc("TRN2", debug=False)

    at_d = nc.dram_tensor("at_in", [128, KC, M_LOC], FP8, kind="ExternalInput").ap()
    bt_d = nc.dram_tensor("bt_in", [NTT, 128, KC, 1024], FP8, kind="ExternalInput").ap()

    colsum_d = nc.dram_tensor("colsum_out", [128, N], BF16, kind="ExternalOutput").ap()
    rowr_d = nc.dram_tensor("rowr_out", [128, MT * NG], F32, kind="ExternalOutput").ap()

    with ExitStack() as ctx:
        tc = ctx.enter_context(tile.TileContext(nc))
        singles = ctx.enter_context(tc.tile_pool(name="singles", bufs=1))
        btp = ctx.enter_context(tc.tile_pool(name="btp", bufs=NTT))
        expp = ctx.enter_context(tc.tile_pool(name="expp", bufs=6))
        psum = ctx.enter_context(tc.tile_pool(name="psum", bufs=2, space="PSUM"))

        at_t = singles.tile([128, KC, M_LOC], FP8)
        bt_tiles = [
            btp.tile([128, KC, 1024], FP8, name=f"bt{t}", tag="bt")
            for t in range(NTT)
        ]
        # Parallel first loads across three DMA queues (sync/scalar/gpsimd)
        # so group 0's operands (at, bt0, bt1) land ASAP; rest alternates
        # between the sync and gpsimd queues.
        nc.sync.dma_start(at_t[:, 0:2, :], at_d[:, 0:2, :])
        nc.scalar.dma_start(bt_tiles[0][:, 0:2, :], bt_d[0, :, 0:2, :])
        nc.gpsimd.dma_start(bt_tiles[1][:, 0:2, :], bt_d[1, :, 0:2, :])
        nc.sync.dma_start(at_t[:, 2:4, :], at_d[:, 2:4, :])
        nc.scalar.dma_start(bt_tiles[0][:, 2:4, :], bt_d[0, :, 2:4, :])
        nc.gpsimd.dma_start(bt_tiles[1][:, 2:4, :], bt_d[1, :, 2:4, :])
        for t in range(2, NTT):
            eng = nc.sync if t % 2 == 0 else nc.gpsimd
            eng.dma_start(bt_tiles[t], bt_d[t])



        bias_t = singles.tile([128, 1], F32)
        nc.vector.memset(bias_t, -shift)
        colacc_a = singles.tile([128, N], BF16)
        colacc_b = singles.tile([128, N], BF16)
        if ROWMODE == "ttr":
            nc.vector.memset(colacc_b, 0.0)
        rowr_sb = singles.tile([128, MT * NG], F32)

        def emit_colacc(pend):
            mt, g, ridx, e_ap, act_step = pend
            dst, src = (
                (colacc_a, colacc_b) if mt % 2 == 0 else (colacc_b, colacc_a)
            )
            if ROWMODE == "ttr":
                nc.vector.tensor_tensor_reduce(
                    out=dst[:, ts(g, 2048)],
                    in0=src[:, ts(g, 2048)],
                    in1=e_ap,
                    scale=1.0,
                    scalar=0.0,
                    op0=mybir.AluOpType.add,
                    op1=mybir.AluOpType.add,
                    accum_out=rowr_sb[:, ridx : ridx + 1],
                )
            elif not act_step:
                # out = (e * s) + src ; accum_out = sum(out) = prefix R[mt,g]
                # mt==0: out = (e * 0) + e  (no src needed, no memset)
                nc.vector.scalar_tensor_tensor(
                    out=dst[:, ts(g, 2048)],
                    in0=e_ap,
                    scalar=1.0 if mt > 0 else 0.0,
                    in1=src[:, ts(g, 2048)] if mt > 0 else e_ap,
                    op0=mybir.AluOpType.mult,
                    op1=mybir.AluOpType.add,
                    accum_out=rowr_sb[:, ridx : ridx + 1],
                )
            elif mt == 0:
                nc.vector.tensor_copy(dst[:, ts(g, 2048)], e_ap)
            else:
                nc.vector.tensor_add(dst[:, ts(g, 2048)], src[:, ts(g, 2048)], e_ap)
            if mt == MT - 1:
                nc.sync.dma_start(colsum_d[:, ts(g, 2048)], dst[:, ts(g, 2048)])

        pending = None
        for mt in range(MT):
            for g in range(NG):
                s_ps = psum.tile([128, 2048], F32, name=f"s{mt}_{g}", tag="spsum")
                if USE_DR:
                    for kc2 in range(2):
                        for b in range(4):
                            t, h = 2 * g + b // 2, b % 2
                            nc.tensor.matmul(
                                s_ps[:, ts(b, 512)],
                                at_t[:, 2 * kc2 : 2 * kc2 + 2, ts(mt, 128)],
                                bt_tiles[t][:, 2 * kc2 : 2 * kc2 + 2, ts(h, 512)],
                                start=(kc2 == 0),
                                stop=(kc2 == 1),
                                perf_mode=mybir.MatmulPerfMode.DoubleRow,
                            )
                else:
                    for kc in range(KC):
                        for b in range(4):
                            t, h = 2 * g + b // 2, b % 2
                            nc.tensor.matmul(
                                s_ps[:, ts(b, 512)],
                                at_t[:, kc, ts(mt, 128)],
                                bt_tiles[t][:, kc, ts(h, 512)],
                                start=(kc == 0),
                                stop=(kc == KC - 1),
                            )
                ridx = mt * NG + g
                act_step = _is_act_step(ridx)
                if _is_trick(ridx):
                    # pure-float exp2 bit trick: y = s*a + (b + 1.5*2^23) in
                    # f32; RNE snaps y to an integer, whose low 16 bits equal
                    # round(s*a + b) = the bf16 bit pattern of exp(l - shift).
                    e_f = expp.tile(
                        [128, 2048, 2], BF16, name=f"e{mt}_{g}", tag="exp"
                    )
                    nc.vector.tensor_scalar(
                        out=e_f.bitcast(F32).squeeze(2),
                        in0=s_ps,
                        scalar1=trick_a,
                        scalar2=trick_b + 12582912.0,  # + 1.5 * 2^23
                        op0=mybir.AluOpType.mult,
                        op1=mybir.AluOpType.add,
                    )
                    e_t = e_f[:, :, 0]  # low halves, stride 2
                else:
                    e_t = expp.tile([128, 2048], BF16, name=f"e{mt}_{g}", tag="exp")
                    nc.scalar.activation(
                        e_t,
                        s_ps,
                        mybir.ActivationFunctionType.Exp,
                        bias=bias_t,
                        scale=act_scale,
                        accum_out=(
                            rowr_sb[:, ridx : ridx + 1] if act_step else None
                        ),
                    )
                if pending is not None:
                    emit_colacc(pending)
                pending = (mt, g, ridx, e_t, act_step)
        emit_colacc(pending)
        nc.sync.dma_start(rowr_d, rowr_sb)

    nc.compile()
    return nc


def _prep_inputs(img, txt):
    imgq = (FSCALE * img).astype(NP_FP8)
    txtq = (FSCALE * txt).astype(NP_FP8)
    in_maps = []
    for c in range(NC):
        A = imgq[c * M_LOC : (c + 1) * M_LOC]                   # [1024, 512] fp8
        at = np.ascontiguousarray(
            A.T.reshape(KC, 128, M_LOC).transpose(1, 0, 2)
        )                                                       # [128, 4, 1024]
        tr = np.roll(txtq, -c * M_LOC, axis=0)                  # local col j -> global (j + c*1024) % N
        bt = np.ascontiguousarray(
            tr.T.reshape(KC, 128, NTT, 1024).transpose(2, 1, 0, 3)
        )                                                       # [8, 128, 4, 1024]
        in_maps.append({"at_in": at, "bt_in": bt})
    return in_maps


def kernel(image_features, text_features, logit_scale):
    global LAST_RESULTS
    img = np.ascontiguousarray(np.asarray(image_features, dtype=np.float32))
    txt = np.ascontiguousarray(np.asarray(text_features, dtype=np.float32))
    scale = float(np.asarray(logit_scale))
    shift = 0.5 * scale

    if scale not in _CACHE:
        _CACHE[scale] = _build(scale, shift)
    nc = _CACHE[scale]

    in_maps = _prep_inputs(img, txt)
    res = run_bass_kernel_spmd(nc, in_maps, core_ids=list(range(NC)))
    LAST_RESULTS = res

    # exact diagonal in f64 (independent of the fp8 matmul path)
    diag = scale * np.einsum(
        "ij,ij->i", img.astype(np.float64), txt.astype(np.float64)
    )

    colsum_tot = np.zeros(N, dtype=np.float64)
    lse_rows = []
    for c, r in enumerate(res.results):
        colacc = r["colsum_out"].astype(np.float64)             # [128, N]
        colsum_tot += np.roll(colacc.sum(axis=0), c * M_LOC)
        R = r["rowr_out"].astype(np.float64).reshape(128, MT, NG)
        # act steps: R[mt,g] is the (mt,g) chunk directly.
        # stt steps: R[mt,g] is the prefix sum_{m<=mt} chunk[m,g]; recover
        # sequentially (works for any act/stt interleaving per g).
        chunks = np.empty_like(R)
        prefix = np.zeros((128, NG), dtype=np.float64)
        for mt in range(MT):
            for g in range(NG):
                if _is_act_step(mt * NG + g):
                    chunks[:, mt, g] = R[:, mt, g]
                else:
                    # measured bf16 prefix minus f32-exact prefix can go
                    # slightly negative for tiny chunks; true chunks are >= 0
                    chunks[:, mt, g] = np.maximum(R[:, mt, g] - prefix[:, g], 0.0)
                prefix[:, g] += chunks[:, mt, g]
        rowsum = chunks.sum(axis=2)                             # [p, mt]
        lse_rows.append(shift + np.log(rowsum.T.reshape(-1)))   # row = mt*128 + p
    lse_row = np.concatenate(lse_rows)
    lse_col = shift + np.log(colsum_tot)

    loss = 0.5 * (np.mean(lse_row - diag) + np.mean(lse_col - diag))
    return np.float32(loss)


# revision 43
# speedup vs baseline: 1.0500x; 1.0270x over previous
"""CLIP contrastive loss on 8 Trainium2 NeuronCores (Bass/Tile), fp8 edition.

Strategy (data-parallel over image rows, hint's local_loss path):
  - Core c holds image rows [c*1024, (c+1)*1024) and the FULL text matrix.
  - Text rows are rolled by c*1024 on the host so every core's diagonal
    block sits at local column 0 (the compiled program is core-independent).
  - Features are pre-scaled by 16 and quantized to fp8 e4m3 on the host;
    matmuls run in DoubleRow perf mode (2 fp8 MACs/cell/cycle), K=512 as
    two K=256 DoubleRow accumulation steps.  Loop order is mt-outer /
    column-group-inner so the stationary (image) operand is reused across
    8 matmuls per LDWEIGHTS.
  - PSUM is split in two 4-bank groups [128,2048] f32 (ping-pong).  For
    each finished group, ScalarE does one exp activation (scale=100/256,
    bias=-shift) PSUM->SBUF bf16.
  - VectorE folds each exp tile into a per-column-group accumulator with
    tensor_tensor_reduce; its free per-partition accum output yields the
    running (prefix over mt) sum R[mt,g] = sum_j colacc[p, g-cols].  The
    host recovers per-(mt,g) row-sum chunks by differencing R along mt --
    no ACTIVATION_READ_ACCUMULATOR instructions needed.
  - Host: diag computed exactly in f64 (tiny einsum); col sums reduced
    over partitions (+roll); row sums from R diffs; loss in f64.

Fixed-shift logsumexp is numerically safe: logits are bounded by +-scale
and shift = scale/2 keeps every term that matters in normal f32 range.
"""

from contextlib import ExitStack

import numpy as np
import ml_dtypes

import concourse.bass as bass
from concourse import bacc
import concourse.tile as tile
from concourse import mybir
from concourse.bass import ts
from concourse.bass_utils import run_bass_kernel_spmd

N = 8192
D = 512
NC = 8
M_LOC = N // NC          # 1024 image rows per core
MT = M_LOC // 128        # 8 m-tiles of 128 rows
NTT = N // 1024          # 8 text tiles of 1024 cols
NG = N // 2048           # 4 column groups of 2048 cols (one 4-bank PSUM set)
KC = D // 128            # 4 contraction chunks of 128
FSCALE = 16.0            # host feature pre-scale before fp8 quantization

F32 = mybir.dt.float32
BF16 = mybir.dt.bfloat16
FP8 = mybir.dt.float8e4
NP_FP8 = ml_dtypes.float8_e4m3   # IEEE e4m3: max 240, matches TRN FP8_EXP4

import os

USE_DR = os.environ.get("KMM_DR", "1") == "1"  # DoubleRow fp8 matmuls
# rowsum strategy: "act" = activation accum_out (safe),
# "ttr" = tensor_tensor_reduce prefix trick, "stt" = scalar_tensor_tensor prefix,
# "hybrid" = mix of act (direct chunks) and stt (prefix R) balancing ACT vs DVE
ROWMODE = os.environ.get("KROW", "hybrid")


TRICK_SET = (2, 7, 13, 18, 23, 27)  # spread out, none in mt=7 (shorter tail)


def _is_trick(ridx: int) -> bool:
    # groups whose exp runs on DVE via the exp2 bit-trick (frees ScalarE)
    return ROWMODE == "hybrid" and ridx in TRICK_SET


def _is_act_step(ridx: int) -> bool:
    if ROWMODE == "act":
        return True
    if ROWMODE in ("ttr", "stt"):
        return False
    return not _is_trick(ridx)  # hybrid: ACT accum everywhere except trick groups


LOG2E = 1.4426950408889634
C_MAGIC = 0.05753  # exp2 linear-interp magic, calibrated mean-neutral (RNE)

_CACHE = {}
LAST_RESULTS = None


def _build(scale: float, shift: float):
    act_scale = scale / (FSCALE * FSCALE)
    # DVE exp2 bit-trick: i16 bits = round(s*trick_a + trick_b) viewed as bf16
    # give 2^((l - shift)*log2e) = exp(l - shift), l = act_scale * s
    trick_a = 128.0 * LOG2E * act_scale
    trick_b = 128.0 * (127.0 - C_MAGIC - shift * LOG2E)
    nc = bacc.Bacc("TRN2", debug=False)

    at_d = nc.dram_tensor("at_in", [128, KC, M_LOC], FP8, kind="ExternalInput").ap()
    bt_d = nc.dram_tensor("bt_in", [NTT, 128, KC, 1024], FP8, kind="ExternalInput").ap()

    colsum_d = nc.dram_tensor("colsum_out", [128, N], BF16, kind="ExternalOutput").ap()
    e7_d = nc.dram_tensor("e7_out", [NG, 128, 2048], BF16, kind="ExternalOutput").ap()
    rowr_d = nc.dram_tensor("rowr_out", [128, MT * NG], F32, kind="ExternalOutput").ap()

    with ExitStack() as ctx:
        tc = ctx.enter_context(tile.TileContext(nc))
        singles = ctx.enter_context(tc.tile_pool(name="singles", bufs=1))
        btp = ctx.enter_context(tc.tile_pool(name="btp", bufs=NTT))
        expp = ctx.enter_context(tc.tile_pool(name="expp", bufs=6))
        psum = ctx.enter_context(tc.tile_pool(name="psum", bufs=2, space="PSUM"))

        at_t = singles.tile([128, KC, M_LOC], FP8)
        bt_tiles = [
            btp.tile([128, KC, 1024], FP8, name=f"bt{t}", tag="bt")
            for t in range(NTT)
        ]
        # Parallel first loads across three DMA queues (sync/scalar/gpsimd)
        # so group 0's operands (at, bt0, bt1) land ASAP; rest alternates
        # between the sync and gpsimd queues.
        nc.sync.dma_start(at_t[:, 0:2, :], at_d[:, 0:2, :])
        nc.scalar.dma_start(bt_tiles[0][:, 0:2, :], bt_d[0, :, 0:2, :])
        nc.gpsimd.dma_start(bt_tiles[1][:, 0:2, :], bt_d[1, :, 0:2, :])
        nc.sync.dma_start(at_t[:, 2:4, :], at_d[:, 2:4, :])
        nc.scalar.dma_start(bt_tiles[0][:, 2:4, :], bt_d[0, :, 2:4, :])
        nc.gpsimd.dma_start(bt_tiles[1][:, 2:4, :], bt_d[1, :, 2:4, :])
        for t in range(2, NTT):
            eng = nc.sync if t % 2 == 0 else nc.gpsimd
            eng.dma_start(bt_tiles[t], bt_d[t])



        bias_t = singles.tile([128, 1], F32)
        nc.vector.memset(bias_t, -shift)
        colacc_a = singles.tile([128, N], BF16)
        colacc_b = singles.tile([128, N], BF16)
        if ROWMODE == "ttr":
            nc.vector.memset(colacc_b, 0.0)
        rowr_sb = singles.tile([128, MT * NG], F32)

        def emit_colacc(pend):
            mt, g, ridx, e_ap, act_step = pend
            dst, src = (
                (colacc_a, colacc_b) if mt % 2 == 0 else (colacc_b, colacc_a)
            )
            if mt == MT - 1:
                # final m-tile: ship the exp tile itself (host adds its column
                # sums) so the tail ends at the last activation, not a
                # colacc-add + 2MB DMA chain.  Its row chunk comes from the
                # ACT accumulator (mt=7 groups are never trick groups).
                nc.sync.dma_start(e7_d[g], e_ap)
                return
            if ROWMODE == "ttr":
                nc.vector.tensor_tensor_reduce(
                    out=dst[:, ts(g, 2048)],
                    in0=src[:, ts(g, 2048)],
                    in1=e_ap,
                    scale=1.0,
                    scalar=0.0,
                    op0=mybir.AluOpType.add,
                    op1=mybir.AluOpType.add,
                    accum_out=rowr_sb[:, ridx : ridx + 1],
                )
            elif not act_step:
                # out = (e * s) + src ; accum_out = sum(out) = prefix R[mt,g]
                # mt==0: out = (e * 0) + e  (no src needed, no memset)
                nc.vector.scalar_tensor_tensor(
                    out=dst[:, ts(g, 2048)],
                    in0=e_ap,
                    scalar=1.0 if mt > 0 else 0.0,
                    in1=src[:, ts(g, 2048)] if mt > 0 else e_ap,
                    op0=mybir.AluOpType.mult,
                    op1=mybir.AluOpType.add,
                    accum_out=rowr_sb[:, ridx : ridx + 1],
                )
            elif mt == 0:
                nc.vector.tensor_copy(dst[:, ts(g, 2048)], e_ap)
            else:
                nc.vector.tensor_add(dst[:, ts(g, 2048)], src[:, ts(g, 2048)], e_ap)
            if mt == MT - 2:
                # prefix through mt=6 goes out now, overlapping mt=7 compute
                nc.sync.dma_start(colsum_d[:, ts(g, 2048)], dst[:, ts(g, 2048)])

        pending = None
        for mt in range(MT):
            for g in range(NG):
                s_ps = psum.tile([128, 2048], F32, name=f"s{mt}_{g}", tag="spsum")
                if USE_DR:
                    for kc2 in range(2):
                        for b in range(4):
                            t, h = 2 * g + b // 2, b % 2
                            nc.tensor.matmul(
                                s_ps[:, ts(b, 512)],
                                at_t[:, 2 * kc2 : 2 * kc2 + 2, ts(mt, 128)],
                                bt_tiles[t][:, 2 * kc2 : 2 * kc2 + 2, ts(h, 512)],
                                start=(kc2 == 0),
                                stop=(kc2 == 1),
                                perf_mode=mybir.MatmulPerfMode.DoubleRow,
                            )
                else:
                    for kc in range(KC):
                        for b in range(4):
                            t, h = 2 * g + b // 2, b % 2
                            nc.tensor.matmul(
                                s_ps[:, ts(b, 512)],
                                at_t[:, kc, ts(mt, 128)],
                                bt_tiles[t][:, kc, ts(h, 512)],
                                start=(kc == 0),
                                stop=(kc == KC - 1),
                            )
                ridx = mt * NG + g
                act_step = _is_act_step(ridx)
                if _is_trick(ridx):
                    # pure-float exp2 bit trick: y = s*a + (b + 1.5*2^23) in
                    # f32; RNE snaps y to an integer, whose low 16 bits equal
                    # round(s*a + b) = the bf16 bit pattern of exp(l - shift).
                    e_f = expp.tile(
                        [128, 2048, 2], BF16, name=f"e{mt}_{g}", tag="exp"
                    )
                    nc.vector.tensor_scalar(
                        out=e_f.bitcast(F32).squeeze(2),
                        in0=s_ps,
                        scalar1=trick_a,
                        scalar2=trick_b + 12582912.0,  # + 1.5 * 2^23
                        op0=mybir.AluOpType.mult,
                        op1=mybir.AluOpType.add,
                    )
                    e_t = e_f[:, :, 0]  # low halves, stride 2
                else:
                    e_t = expp.tile([128, 2048], BF16, name=f"e{mt}_{g}", tag="exp")
                    nc.scalar.activation(
                        e_t,
                        s_ps,
                        mybir.ActivationFunctionType.Exp,
                        bias=bias_t,
                        scale=act_scale,
                        accum_out=(
                            rowr_sb[:, ridx : ridx + 1] if act_step else None
                        ),
                    )
                if pending is not None:
                    emit_colacc(pending)
                pending = (mt, g, ridx, e_t, act_step)
        emit_colacc(pending)
        nc.sync.dma_start(rowr_d, rowr_sb)

    nc.compile()
    return nc


def _prep_inputs(img, txt):
    imgq = (FSCALE * img).astype(NP_FP8)
    txtq = (FSCALE * txt).astype(NP_FP8)
    in_maps = []
    for c in range(NC):
        A = imgq[c * M_LOC : (c + 1) * M_LOC]                   # [1024, 512] fp8
        at = np.ascontiguousarray(
            A.T.reshape(KC, 128, M_LOC).transpose(1, 0, 2)
        )                                                       # [128, 4, 1024]
        tr = np.roll(txtq, -c * M_LOC, axis=0)                  # local col j -> global (j + c*1024) % N
        bt = np.ascontiguousarray(
            tr.T.reshape(KC, 128, NTT, 1024).transpose(2, 1, 0, 3)
        )                                                       # [8, 128, 4, 1024]
        in_maps.append({"at_in": at, "bt_in": bt})
    return in_maps


def kernel(image_features, text_features, logit_scale):
    global LAST_RESULTS
    img = np.ascontiguousarray(np.asarray(image_features, dtype=np.float32))
    txt = np.ascontiguousarray(np.asarray(text_features, dtype=np.float32))
    scale = float(np.asarray(logit_scale))
    shift = 0.5 * scale

    if scale not in _CACHE:
        _CACHE[scale] = _build(scale, shift)
    nc = _CACHE[scale]

    in_maps = _prep_inputs(img, txt)
    res = run_bass_kernel_spmd(nc, in_maps, core_ids=list(range(NC)))
    LAST_RESULTS = res

    # exact diagonal in f64 (independent of the fp8 matmul path)
    diag = scale * np.einsum(
        "ij,ij->i", img.astype(np.float64), txt.astype(np.float64)
    )

    colsum_tot = np.zeros(N, dtype=np.float64)
    lse_rows = []
    for c, r in enumerate(res.results):
        colacc = r["colsum_out"].astype(np.float64)             # [128, N] (mt<=6)
        e7 = r["e7_out"].astype(np.float64)                     # [NG, 128, 2048]
        local_col = colacc.sum(axis=0) + e7.sum(axis=1).reshape(-1)
        colsum_tot += np.roll(local_col, c * M_LOC)
        R = r["rowr_out"].astype(np.float64).reshape(128, MT, NG)
        # act steps: R[mt,g] is the (mt,g) chunk directly.
        # stt steps: R[mt,g] is the prefix sum_{m<=mt} chunk[m,g]; recover
        # sequentially (works for any act/stt interleaving per g).
        chunks = np.empty_like(R)
        prefix = np.zeros((128, NG), dtype=np.float64)
        for mt in range(MT):
            for g in range(NG):
                if _is_act_step(mt * NG + g):
                    chunks[:, mt, g] = R[:, mt, g]
                else:
                    # measured bf16 prefix minus f32-exact prefix can go
                    # slightly negative for tiny chunks; true chunks are >= 0
                    chunks[:, mt, g] = np.maximum(R[:, mt, g] - prefix[:, g], 0.0)
                prefix[:, g] += chunks[:, mt, g]
        rowsum = chunks.sum(axis=2)                             # [p, mt]
        lse_rows.append(shift + np.log(rowsum.T.reshape(-1)))   # row = mt*128 + p
    lse_row = np.concatenate(lse_rows)
    lse_col = shift + np.log(colsum_tot)

    loss = 0.5 * (np.mean(lse_row - diag) + np.mean(lse_col - diag))
    return np.float32(loss)
